# revision 3
# baseline (speedup 1.0000x reference)
"""Trainium2 Bass kernel for the LSTM decoder — v4: latency-optimized
two-recurrence schedule.

Per core (256 batch rows) the batch is split into two independent 128-column
recurrences (X, Y) phase-shifted by half a step.  v4 restructures v3 to cut
the per-half critical path (which set the v3 period):

- PSUM bank order [i, g, f, o]: one merged sigmoid ACT over [0:1536] covers
  every gate the DVE c-chain needs (i, g, f); sigma(o) is a separate ACT that
  only gates the h-write.  Matmuls are emitted igf-banks first, o-bank after,
  so the big sigmoid starts after 48 of 64 gate matmuls.
- DVE chain uses fused scalar_tensor_tensor (4x DVE mode):
    t1 = (u_g - 0.5) * u_i          # tanh(g)=2*sigmoid(2g)-1 folded
    t2 = (ct * 1) * u_f
    ct = (t1 * 2) + t2
  3 ops instead of 4, each ~194ns instead of ~327ns.
- h8 (fp8 h for next-step matmuls) written as two [128, 256] chunks so the
  kc01 matmuls of the next step start one chunk earlier; h16 (fp16 h for y)
  is a cheap STT after; y_copy runs on the idle Pool engine.
- PE emission order is ready-time monotonic so the 4-deep wait queue never
  traps ready matmuls behind blocked ones:
    igf(h0,t) o(h0,t) | y(h1,t-2) bias(h1,t) | igf(h1,t) o(h1,t) |
    y(h0,t-1) bias(h0,t+1)
- fp8 (e4m3) DoubleRow matmuls: two K-slots carry the (hi, lo) split of the
  merged W = W_ih + W_hh (valid since the output h is fed back as the next
  input); moving operand is the plain fp8 h chunk broadcast into both slots.
- y_t = W_d @ h_t (fp16) lands in the o-bank s0 PSUM region after sigma(o)
  reads it; scaled and bias-added on the host.
"""

import numpy as np
import ml_dtypes
from contextlib import ExitStack

import concourse.bacc as bacc
import concourse.mybir as mybir
from concourse import tile
from concourse.bass_utils import run_bass_kernel_spmd

fp32 = mybir.dt.float32
fp16 = mybir.dt.float16
fp8 = mybir.dt.float8e4
F8 = ml_dtypes.float8_e4m3fn
AF = mybir.ActivationFunctionType
ALU = mybir.AluOpType
DR = mybir.MatmulPerfMode.DoubleRow

P = 128
B = 256          # batch rows per core
HW = 128         # half-batch width
HC = 4           # hidden chunks of 128
NT = 16          # gate tiles per half
PH = 32
NCORES = 8
SP = 1024.0      # weight/bias scale (keeps all fp8 <= 240: IEEE-e4m3 safe)

# gate-class order in PSUM banks: [i, g, f, o]; W row bases (torch i,f,g,o)
RB = [0, 1024, 512, 1536]

_CACHE = {}


def _build():
    nc = bacc.Bacc("TRN2", target_bir_lowering=False, debug=False,
                   num_devices=NCORES)

    ws_d = nc.dram_tensor("ws", [P, HC, 2, NT, P], fp8, kind="ExternalInput")
    wih_d = nc.dram_tensor("wih", [P, HC, 2, NT, P], fp8, kind="ExternalInput")
    bs_d = nc.dram_tensor("bs", [1, 2, NT, P], fp8, kind="ExternalInput")
    wd_d = nc.dram_tensor("wd", [P, HC, 2], fp16, kind="ExternalInput")
    zhi_d = nc.dram_tensor("zhi", [P, HC, B], fp8, kind="ExternalInput")
    zlo_d = nc.dram_tensor("zlo", [P, HC, B], fp8, kind="ExternalInput")
    ones_d = nc.dram_tensor("ones", [1, HW], fp8, kind="ExternalInput")
    y_d = nc.dram_tensor("y", [2, PH * B], fp32, kind="ExternalOutput")

    with tile.TileContext(nc) as tc:
        with ExitStack() as ctx:
            const = ctx.enter_context(tc.tile_pool(name="const", bufs=1))
            state = ctx.enter_context(tc.tile_pool(name="state", bufs=1))
            pp = ctx.enter_context(tc.tile_pool(name="pp", bufs=1,
                                                space="PSUM"))

            wih = const.tile([P, HC, 2, NT, P], fp8)
            for kc in range(HC):
                nc.sync.dma_start(wih[:, kc], wih_d[:, kc])
            zhi = const.tile([P, HC, B], fp8)
            zlo = const.tile([P, HC, B], fp8)
            nc.sync.dma_start(zhi[:], zhi_d[:])
            nc.sync.dma_start(zlo[:], zlo_d[:])
            bs = const.tile([1, 2, NT, P], fp8)
            nc.sync.dma_start(bs[:], bs_d[:])
            ones = const.tile([1, HW], fp8)
            nc.sync.dma_start(ones[:], ones_d[:])
            wd = const.tile([P, HC, 2], fp16)
            nc.sync.dma_start(wd[:], wd_d[:])
            ws = const.tile([P, HC, 2, NT, P], fp8)
            for kc in range(HC):
                nc.sync.dma_start(ws[:, kc], ws_d[:, kc])

            pH = [pp.tile([P, 2048], fp32, tag=f"p{h}", name=f"p{h}")
                  for h in range(2)]
            u = [state.tile([P, 2048], fp16, tag=f"u{h}", name=f"u{h}")
                 for h in range(2)]
            ct = [state.tile([P, 512], fp16, tag=f"c{h}", name=f"c{h}")
                  for h in range(2)]
            tct = [state.tile([P, 512], fp16, tag=f"tc{h}", name=f"tc{h}")
                   for h in range(2)]
            t1 = [state.tile([P, 512], fp16, tag=f"t1{h}", name=f"t1{h}")
                  for h in range(2)]
            t2 = [state.tile([P, 512], fp16, tag=f"t2{h}", name=f"t2{h}")
                  for h in range(2)]
            h8 = [[state.tile([P, HC * HW], fp8, tag=f"h8{h}b{b}",
                              name=f"h8{h}b{b}") for b in range(2)]
                  for h in range(2)]
            h16 = [[state.tile([P, HC * HW], fp16, tag=f"h16{h}b{b}",
                               name=f"h16{h}b{b}") for b in range(2)]
                   for h in range(2)]
            y_sb = const.tile([2, PH * B], fp32)
            ones_b = ones[:].unsqueeze(1).broadcast_to([1, 2, HW])

            def mov(src_ap):
                return src_ap.unsqueeze(1).broadcast_to([P, 2, HW])

            def tau(beta, s):
                return 4 * beta + s

            def out_ap(h, beta, s):
                c0 = 512 * beta + HW * s
                return pH[h][:, c0:c0 + HW]

            def bias_mm(h, beta, s):
                nc.tensor.matmul(out_ap(h, beta, s),
                                 bs[0:1, :, tau(beta, s), :], ones_b,
                                 start=True if s == 0 else False,
                                 stop=False, perf_mode=DR)

            def kg_mm(h, beta, s, kc, W, src_ap, stop):
                nc.tensor.matmul(out_ap(h, beta, s),
                                 W[:, kc, :, tau(beta, s), :], mov(src_ap),
                                 start=False, stop=stop, perf_mode=DR)

            def hsrc(h, t, kc):
                return h8[h][t % 2][:, kc * HW:(kc + 1) * HW]

            def bias_all(h):
                for beta in range(4):
                    for s in range(HC):
                        bias_mm(h, beta, s)

            def kg_igf(h, t):
                # i, g, f banks (betas 0,1,2), kc01 then kc23: the merged
                # sigmoid needs only these 48 matmuls
                for kcp in ((0, 1), (2, 3)):
                    for kc in kcp:
                        for beta in range(3):
                            for s in range(HC):
                                kg_mm(h, beta, s, kc, ws, hsrc(h, t - 1, kc),
                                      stop=(kc == 3 and s == HC - 1))

            def kg_o(h, t):
                for kc in range(HC):
                    for s in range(HC):
                        kg_mm(h, 3, s, kc, ws, hsrc(h, t - 1, kc),
                              stop=(kc == 3 and s == HC - 1))

            def y_mm(h, t):
                # fp16 matmul from the fp16 h copy (fp8 h noise is too big
                # for y at early steps); lands in o-bank s0 after sigma(o)
                # reads it
                out = pH[h][0:2, 1536:1536 + HW]
                for kc in range(HC):
                    nc.tensor.matmul(out, wd[:, kc, :],
                                     h16[h][t % 2][:, kc * HW:(kc + 1) * HW],
                                     start=(kc == 0), stop=(kc == 3))

            def y_copy(h, t):
                nc.gpsimd.tensor_copy(y_sb[:, B * t + HW * h:
                                           B * t + HW * (h + 1)],
                                      pH[h][0:2, 1536:1536 + HW])

            def step0_mm(h):
                # x = z (two-term), f-gate (beta=2) dead since c0 = 0
                for beta in (0, 1, 3):
                    for s in range(HC):
                        bias_mm(h, beta, s)
                for kc in range(HC):
                    zh = zhi[:, kc, HW * h:HW * (h + 1)]
                    zl = zlo[:, kc, HW * h:HW * (h + 1)]
                    for beta in (0, 1, 3):
                        for s in range(HC):
                            kg_mm(h, beta, s, kc, wih, zh, stop=False)
                            kg_mm(h, beta, s, kc, wih, zl,
                                  stop=(kc == 3 and beta == 3 and s == HC - 1))

            def act_sig(h, t):
                if t == 0:
                    nc.scalar.activation(u[h][:, 0:1024], pH[h][:, 0:1024],
                                         AF.Sigmoid, scale=1.0 / SP)
                else:
                    nc.scalar.activation(u[h][:, 0:1536], pH[h][:, 0:1536],
                                         AF.Sigmoid, scale=1.0 / SP)
                nc.scalar.activation(u[h][:, 1536:2048], pH[h][:, 1536:2048],
                                     AF.Sigmoid, scale=1.0 / SP)

            def dve_c(h, t):
                # t1 = (u_g - 0.5) * u_i   [= sigmoid(i)*tanh(g)/2]
                nc.vector.scalar_tensor_tensor(
                    t1[h][:], u[h][:, 512:1024], 0.5, u[h][:, 0:512],
                    ALU.subtract, ALU.mult)
                if t == 0:
                    nc.vector.tensor_scalar(ct[h][:], t1[h][:], 2.0, 0.0,
                                            ALU.mult, ALU.subtract)
                else:
                    # t2 = ct * u_f ; ct = 2*t1 + t2
                    nc.vector.scalar_tensor_tensor(
                        t2[h][:], ct[h][:], 1.0, u[h][:, 1024:1536],
                        ALU.mult, ALU.mult)
                    nc.vector.scalar_tensor_tensor(
                        ct[h][:], t1[h][:], 2.0, t2[h][:],
                        ALU.mult, ALU.add)

            def tanh_h(h):
                nc.scalar.activation(tct[h][:], ct[h][:], AF.Tanh)

            def hmuls(h, t):
                hb = h8[h][t % 2]
                nc.vector.tensor_mul(hb[:, 0:256], u[h][:, 1536:1792],
                                     tct[h][:, 0:256])
                nc.vector.tensor_mul(hb[:, 256:512], u[h][:, 1792:2048],
                                     tct[h][:, 256:512])
                nc.vector.scalar_tensor_tensor(
                    h16[h][t % 2][:], tct[h][:], 1.0, u[h][:, 1536:2048],
                    ALU.mult, ALU.mult)

            def chain(h, t):
                act_sig(h, t)
                dve_c(h, t)
                tanh_h(h)
                hmuls(h, t)

            # --- step 0 ---
            for h in range(2):
                step0_mm(h)
                chain(h, 0)
            bias_all(0)

            # --- steady steps: PE emission is ready-time monotonic ---
            # igf(h0,t) o(h0,t) | y(h1,t-2) bias(h1,t) | igf(h1,t) o(h1,t) |
            # y(h0,t-1) bias(h0,t+1)
            for t in range(1, PH):
                kg_igf(0, t)
                kg_o(0, t)
                chain(0, t)
                if t >= 2:
                    y_mm(1, t - 2)
                    y_copy(1, t - 2)
                bias_all(1)
                kg_igf(1, t)
                kg_o(1, t)
                chain(1, t)
                y_mm(0, t - 1)
                y_copy(0, t - 1)
                if t < PH - 1:
                    bias_all(0)

            # --- drain the y tail ---
            y_mm(1, PH - 2)
            y_copy(1, PH - 2)
            y_mm(0, PH - 1)
            y_copy(0, PH - 1)
            y_mm(1, PH - 1)
            y_copy(1, PH - 1)
            nc.sync.dma_start(y_d[:], y_sb[:])
    nc.compile()
    return nc


def _get_nc():
    if "nc" not in _CACHE:
        _CACHE["nc"] = _build()
    return _CACHE["nc"]


def _enc8(x):
    return np.asarray(F8(np.asarray(x, np.float32)))


def _prep_inputs(z, W_ih, W_hh, b_ih, b_hh, W_d):
    z2 = np.asarray(z, np.float32).reshape(2048, 512)
    W_ih = np.asarray(W_ih, np.float32)
    W_sum = W_ih + np.asarray(W_hh, np.float32)
    bias = (np.asarray(b_ih, np.float32) + np.asarray(b_hh, np.float32))

    def fold_w(W):
        W2 = W * SP
        W2[1024:1536] *= 2.0
        return W2

    W2 = fold_w(W_sum)
    Wih2 = fold_w(W_ih)
    Bp = bias * SP
    Bp[1024:1536] *= 2.0

    # tile tau = 4*beta + s -> W rows RB[beta] + 128*s
    rows = np.empty((NT, P), np.int64)
    for beta in range(4):
        for s in range(HC):
            rows[4 * beta + s] = RB[beta] + 128 * s + np.arange(P)

    def to_ws(W2f):
        hi = _enc8(W2f).astype(np.float32)
        lo = _enc8(W2f - hi).astype(np.float32)

        def lay(Wq):
            a = Wq[rows]                                      # [16,128,512]
            a = a.reshape(NT, P, HC, P).transpose(3, 2, 0, 1)  # [p,kc,tau,m]
            return a
        out = np.stack([lay(hi), lay(lo)], axis=2)             # [p,kc,2,tau,m]
        return np.ascontiguousarray(_enc8(out))

    ws = to_ws(W2)
    wih = to_ws(Wih2)

    bhi = _enc8(Bp).astype(np.float32)
    blo = _enc8(Bp - bhi).astype(np.float32)
    bs = np.stack([bhi[rows], blo[rows]], axis=0)
    bs = np.ascontiguousarray(_enc8(bs[None]))                 # [1,2,16,128]

    Wd2 = np.asarray(W_d, np.float32) * SP
    wd = np.ascontiguousarray(
        Wd2.T.reshape(HC, P, 2).transpose(1, 0, 2)).astype(np.float16)

    ones = _enc8(np.ones((1, HW), np.float32))

    in_maps = []
    for cix in range(NCORES):
        zc = z2[cix * B:(cix + 1) * B].T                       # [512, 256]
        zhi = _enc8(zc).astype(np.float32)
        zlo = _enc8(zc - zhi).astype(np.float32)
        zhi = zhi.reshape(HC, P, B).transpose(1, 0, 2)
        zlo = zlo.reshape(HC, P, B).transpose(1, 0, 2)
        in_maps.append({
            "ws": ws, "wih": wih, "bs": bs, "wd": wd,
            "zhi": np.ascontiguousarray(_enc8(zhi)),
            "zlo": np.ascontiguousarray(_enc8(zlo)),
            "ones": ones,
        })
    return in_maps


def run(inputs, trace=False, **kw):
    nc = _get_nc()
    in_maps = _prep_inputs(inputs["z"], inputs["W_ih"], inputs["W_hh"],
                           inputs["b_ih"], inputs["b_hh"], inputs["W_d"])
    res = run_bass_kernel_spmd(nc, in_maps, core_ids=list(range(NCORES)),
                               trace=trace, **kw)
    b_d = np.asarray(inputs["b_d"], np.float32)
    outs = []
    for cix in range(NCORES):
        arr = res.results[cix]["y"] / SP                       # [2, PH*B]
        outs.append(arr.reshape(2, PH, B).transpose(2, 1, 0))
    y = np.concatenate(outs, axis=0) + b_d[None, None, :]
    return np.ascontiguousarray(y, dtype=np.float32), res


def kernel(**inputs):
    y, _ = run(inputs, trace=False)
    return y


# revision 5
# speedup vs baseline: 1.0276x; 1.0276x over previous
"""Trainium2 Bass kernel for the LSTM decoder — v4: latency-optimized
two-recurrence schedule.

Per core (256 batch rows) the batch is split into two independent 128-column
recurrences (X, Y) phase-shifted by half a step.  v4 restructures v3 to cut
the per-half critical path (which set the v3 period):

- PSUM bank order [i, g, f, o]: one merged sigmoid ACT over [0:1536] covers
  every gate the DVE c-chain needs (i, g, f); sigma(o) is a separate ACT that
  only gates the h-write.  Matmuls are emitted igf-banks first, o-bank after,
  so the big sigmoid starts after 48 of 64 gate matmuls.
- DVE chain uses fused scalar_tensor_tensor (4x DVE mode):
    t1 = (u_g - 0.5) * u_i          # tanh(g)=2*sigmoid(2g)-1 folded
    t2 = (ct * 1) * u_f
    ct = (t1 * 2) + t2
  3 ops instead of 4, each ~194ns instead of ~327ns.
- h8 (fp8 h for next-step matmuls) written as two [128, 256] chunks so the
  kc01 matmuls of the next step start one chunk earlier; h16 (fp16 h for y)
  is a cheap STT after; y_copy runs on the idle Pool engine.
- PE emission order is ready-time monotonic so the 4-deep wait queue never
  traps ready matmuls behind blocked ones:
    igf(h0,t) o(h0,t) | y(h1,t-2) bias(h1,t) | igf(h1,t) o(h1,t) |
    y(h0,t-1) bias(h0,t+1)
- fp8 (e4m3) DoubleRow matmuls: two K-slots carry the (hi, lo) split of the
  merged W = W_ih + W_hh (valid since the output h is fed back as the next
  input); moving operand is the plain fp8 h chunk broadcast into both slots.
- y_t = W_d @ h_t (fp16) lands in the o-bank s0 PSUM region after sigma(o)
  reads it; scaled and bias-added on the host.
"""

import numpy as np
import ml_dtypes
from contextlib import ExitStack

import concourse.bacc as bacc
import concourse.mybir as mybir
from concourse import tile
from concourse.bass_utils import run_bass_kernel_spmd

fp32 = mybir.dt.float32
fp16 = mybir.dt.float16
fp8 = mybir.dt.float8e4
F8 = ml_dtypes.float8_e4m3fn
AF = mybir.ActivationFunctionType
ALU = mybir.AluOpType
DR = mybir.MatmulPerfMode.DoubleRow

P = 128
B = 256          # batch rows per core
HW = 128         # half-batch width
HC = 4           # hidden chunks of 128
NT = 16          # gate tiles per half
PH = 32
NCORES = 8
SP = 1024.0      # weight/bias scale (keeps all fp8 <= 240: IEEE-e4m3 safe)

# gate-class order in PSUM banks: [i, g, f, o]; W row bases (torch i,f,g,o)
RB = [0, 1024, 512, 1536]

_CACHE = {}


def _build():
    nc = bacc.Bacc("TRN2", target_bir_lowering=False, debug=False,
                   num_devices=NCORES)

    ws_d = nc.dram_tensor("ws", [P, HC, 2, NT, P], fp8, kind="ExternalInput")
    wih_d = nc.dram_tensor("wih", [P, HC, 2, NT, P], fp8, kind="ExternalInput")
    bs_d = nc.dram_tensor("bs", [1, 2, NT, P], fp8, kind="ExternalInput")
    wd_d = nc.dram_tensor("wd", [P, HC, 2], fp16, kind="ExternalInput")
    zhi_d = nc.dram_tensor("zhi", [P, HC, B], fp8, kind="ExternalInput")
    zlo_d = nc.dram_tensor("zlo", [P, HC, B], fp8, kind="ExternalInput")
    ones_d = nc.dram_tensor("ones", [1, HW], fp8, kind="ExternalInput")
    y_d = nc.dram_tensor("y", [2, PH * B], fp32, kind="ExternalOutput")

    with tile.TileContext(nc) as tc:
        with ExitStack() as ctx:
            const = ctx.enter_context(tc.tile_pool(name="const", bufs=1))
            state = ctx.enter_context(tc.tile_pool(name="state", bufs=1))
            pp = ctx.enter_context(tc.tile_pool(name="pp", bufs=1,
                                                space="PSUM"))

            wih = const.tile([P, HC, 2, NT, P], fp8)
            for kc in range(HC):
                nc.sync.dma_start(wih[:, kc], wih_d[:, kc])
            zhi = const.tile([P, HC, B], fp8)
            zlo = const.tile([P, HC, B], fp8)
            nc.sync.dma_start(zhi[:], zhi_d[:])
            nc.sync.dma_start(zlo[:], zlo_d[:])
            bs = const.tile([1, 2, NT, P], fp8)
            nc.sync.dma_start(bs[:], bs_d[:])
            ones = const.tile([1, HW], fp8)
            nc.sync.dma_start(ones[:], ones_d[:])
            wd = const.tile([P, HC, 2], fp16)
            nc.sync.dma_start(wd[:], wd_d[:])
            ws = const.tile([P, HC, 2, NT, P], fp8)
            for kc in range(HC):
                nc.sync.dma_start(ws[:, kc], ws_d[:, kc])

            pH = [pp.tile([P, 2048], fp32, tag=f"p{h}", name=f"p{h}")
                  for h in range(2)]
            u = [state.tile([P, 2048], fp16, tag=f"u{h}", name=f"u{h}")
                 for h in range(2)]
            ct = [state.tile([P, 512], fp16, tag=f"c{h}", name=f"c{h}")
                  for h in range(2)]
            tct = [state.tile([P, 512], fp16, tag=f"tc{h}", name=f"tc{h}")
                   for h in range(2)]
            gt = [state.tile([P, 512], fp16, tag=f"gt{h}", name=f"gt{h}")
                  for h in range(2)]
            t1 = [state.tile([P, 512], fp16, tag=f"t1{h}", name=f"t1{h}")
                  for h in range(2)]
            t2 = [state.tile([P, 512], fp16, tag=f"t2{h}", name=f"t2{h}")
                  for h in range(2)]
            h8 = [[state.tile([P, HC * HW], fp8, tag=f"h8{h}b{b}",
                              name=f"h8{h}b{b}") for b in range(2)]
                  for h in range(2)]
            h16 = [[state.tile([P, HC * HW], fp16, tag=f"h16{h}b{b}",
                               name=f"h16{h}b{b}") for b in range(2)]
                   for h in range(2)]
            y_sb = const.tile([2, PH * B], fp32)
            ones_b = ones[:].unsqueeze(1).broadcast_to([1, 2, HW])

            def mov(src_ap):
                return src_ap.unsqueeze(1).broadcast_to([P, 2, HW])

            def tau(beta, s):
                return 4 * beta + s

            def out_ap(h, beta, s):
                c0 = 512 * beta + HW * s
                return pH[h][:, c0:c0 + HW]

            def bias_mm(h, beta, s):
                nc.tensor.matmul(out_ap(h, beta, s),
                                 bs[0:1, :, tau(beta, s), :], ones_b,
                                 start=True if s == 0 else False,
                                 stop=False, perf_mode=DR)

            def kg_mm(h, beta, s, kc, W, src_ap, stop):
                nc.tensor.matmul(out_ap(h, beta, s),
                                 W[:, kc, :, tau(beta, s), :], mov(src_ap),
                                 start=False, stop=stop, perf_mode=DR)

            def hsrc(h, t, kc):
                return h8[h][t % 2][:, kc * HW:(kc + 1) * HW]

            def bias_all(h):
                for beta in range(4):
                    for s in range(HC):
                        bias_mm(h, beta, s)

            def kg_igf(h, t):
                # i, g, f banks (betas 0,1,2), kc01 then kc23: the merged
                # sigmoid needs only these 48 matmuls
                for kcp in ((0, 1), (2, 3)):
                    for kc in kcp:
                        for beta in range(3):
                            for s in range(HC):
                                kg_mm(h, beta, s, kc, ws, hsrc(h, t - 1, kc),
                                      stop=(kc == 3 and s == HC - 1))

            def kg_o(h, t):
                for kc in range(HC):
                    for s in range(HC):
                        kg_mm(h, 3, s, kc, ws, hsrc(h, t - 1, kc),
                              stop=(kc == 3 and s == HC - 1))

            def y_mm(h, t):
                # fp16 matmul from the fp16 h copy (fp8 h noise is too big
                # for y at early steps); lands in o-bank s0 after sigma(o)
                # reads it
                out = pH[h][0:2, 1536:1536 + HW]
                for kc in range(HC):
                    nc.tensor.matmul(out, wd[:, kc, :],
                                     h16[h][t % 2][:, kc * HW:(kc + 1) * HW],
                                     start=(kc == 0), stop=(kc == 3))

            def y_copy(h, t):
                nc.gpsimd.tensor_copy(y_sb[:, B * t + HW * h:
                                           B * t + HW * (h + 1)],
                                      pH[h][0:2, 1536:1536 + HW])

            def step0_mm(h):
                # x = z (two-term), f-gate (beta=2) dead since c0 = 0
                for beta in (0, 1, 3):
                    for s in range(HC):
                        bias_mm(h, beta, s)
                for kc in range(HC):
                    zh = zhi[:, kc, HW * h:HW * (h + 1)]
                    zl = zlo[:, kc, HW * h:HW * (h + 1)]
                    for beta in (0, 1, 3):
                        for s in range(HC):
                            kg_mm(h, beta, s, kc, wih, zh, stop=False)
                            kg_mm(h, beta, s, kc, wih, zl,
                                  stop=(kc == 3 and beta == 3 and s == HC - 1))

            def act_sig(h, t):
                if t == 0:
                    nc.scalar.activation(u[h][:, 0:1024], pH[h][:, 0:1024],
                                         AF.Sigmoid, scale=1.0 / SP)
                else:
                    nc.scalar.activation(u[h][:, 0:1536], pH[h][:, 0:1536],
                                         AF.Sigmoid, scale=1.0 / SP)
                nc.scalar.activation(u[h][:, 1536:2048], pH[h][:, 1536:2048],
                                     AF.Sigmoid, scale=1.0 / SP)

            def dve_c(h, t):
                # gtil = 2*u_g - 1 = tanh(g)  (4x tensor_scalar)
                nc.vector.tensor_scalar(gt[h][:], u[h][:, 512:1024], 2.0, 1.0,
                                        ALU.mult, ALU.subtract)
                if t == 0:
                    nc.vector.tensor_mul(ct[h][:], u[h][:, 0:512], gt[h][:])
                else:
                    nc.vector.tensor_mul(t1[h][:], u[h][:, 0:512], gt[h][:])
                    nc.vector.tensor_mul(t2[h][:], u[h][:, 1024:1536],
                                         ct[h][:])
                    nc.vector.tensor_add(ct[h][:], t1[h][:], t2[h][:])

            def tanh_h(h):
                nc.scalar.activation(tct[h][:], ct[h][:], AF.Tanh)

            def hmuls(h, t):
                hb = h8[h][t % 2]
                nc.vector.tensor_mul(hb[:, 0:256], u[h][:, 1536:1792],
                                     tct[h][:, 0:256])
                nc.vector.tensor_mul(hb[:, 256:512], u[h][:, 1792:2048],
                                     tct[h][:, 256:512])
                nc.vector.tensor_mul(h16[h][t % 2][:], u[h][:, 1536:2048],
                                     tct[h][:])

            def chain(h, t):
                act_sig(h, t)
                dve_c(h, t)
                tanh_h(h)
                hmuls(h, t)

            # --- step 0 ---
            for h in range(2):
                step0_mm(h)
                chain(h, 0)
            bias_all(0)

            # --- steady steps: PE emission is ready-time monotonic ---
            # igf(h0,t) o(h0,t) | y(h1,t-2) bias(h1,t) | igf(h1,t) o(h1,t) |
            # y(h0,t-1) bias(h0,t+1)
            for t in range(1, PH):
                kg_igf(0, t)
                kg_o(0, t)
                chain(0, t)
                if t >= 2:
                    y_mm(1, t - 2)
                    y_copy(1, t - 2)
                bias_all(1)
                kg_igf(1, t)
                kg_o(1, t)
                chain(1, t)
                y_mm(0, t - 1)
                y_copy(0, t - 1)
                if t < PH - 1:
                    bias_all(0)

            # --- drain the y tail ---
            y_mm(1, PH - 2)
            y_copy(1, PH - 2)
            y_mm(0, PH - 1)
            y_copy(0, PH - 1)
            y_mm(1, PH - 1)
            y_copy(1, PH - 1)
            nc.sync.dma_start(y_d[:], y_sb[:])
    nc.compile()
    return nc


def _get_nc():
    if "nc" not in _CACHE:
        _CACHE["nc"] = _build()
    return _CACHE["nc"]


def _enc8(x):
    return np.asarray(F8(np.asarray(x, np.float32)))


def _prep_inputs(z, W_ih, W_hh, b_ih, b_hh, W_d):
    z2 = np.asarray(z, np.float32).reshape(2048, 512)
    W_ih = np.asarray(W_ih, np.float32)
    W_sum = W_ih + np.asarray(W_hh, np.float32)
    bias = (np.asarray(b_ih, np.float32) + np.asarray(b_hh, np.float32))

    def fold_w(W):
        W2 = W * SP
        W2[1024:1536] *= 2.0
        return W2

    W2 = fold_w(W_sum)
    Wih2 = fold_w(W_ih)
    Bp = bias * SP
    Bp[1024:1536] *= 2.0

    # tile tau = 4*beta + s -> W rows RB[beta] + 128*s
    rows = np.empty((NT, P), np.int64)
    for beta in range(4):
        for s in range(HC):
            rows[4 * beta + s] = RB[beta] + 128 * s + np.arange(P)

    def to_ws(W2f):
        hi = _enc8(W2f).astype(np.float32)
        lo = _enc8(W2f - hi).astype(np.float32)

        def lay(Wq):
            a = Wq[rows]                                      # [16,128,512]
            a = a.reshape(NT, P, HC, P).transpose(3, 2, 0, 1)  # [p,kc,tau,m]
            return a
        out = np.stack([lay(hi), lay(lo)], axis=2)             # [p,kc,2,tau,m]
        return np.ascontiguousarray(_enc8(out))

    ws = to_ws(W2)
    wih = to_ws(Wih2)

    bhi = _enc8(Bp).astype(np.float32)
    blo = _enc8(Bp - bhi).astype(np.float32)
    bs = np.stack([bhi[rows], blo[rows]], axis=0)
    bs = np.ascontiguousarray(_enc8(bs[None]))                 # [1,2,16,128]

    Wd2 = np.asarray(W_d, np.float32) * SP
    wd = np.ascontiguousarray(
        Wd2.T.reshape(HC, P, 2).transpose(1, 0, 2)).astype(np.float16)

    ones = _enc8(np.ones((1, HW), np.float32))

    in_maps = []
    for cix in range(NCORES):
        zc = z2[cix * B:(cix + 1) * B].T                       # [512, 256]
        zhi = _enc8(zc).astype(np.float32)
        zlo = _enc8(zc - zhi).astype(np.float32)
        zhi = zhi.reshape(HC, P, B).transpose(1, 0, 2)
        zlo = zlo.reshape(HC, P, B).transpose(1, 0, 2)
        in_maps.append({
            "ws": ws, "wih": wih, "bs": bs, "wd": wd,
            "zhi": np.ascontiguousarray(_enc8(zhi)),
            "zlo": np.ascontiguousarray(_enc8(zlo)),
            "ones": ones,
        })
    return in_maps


def run(inputs, trace=False, **kw):
    nc = _get_nc()
    in_maps = _prep_inputs(inputs["z"], inputs["W_ih"], inputs["W_hh"],
                           inputs["b_ih"], inputs["b_hh"], inputs["W_d"])
    res = run_bass_kernel_spmd(nc, in_maps, core_ids=list(range(NCORES)),
                               trace=trace, **kw)
    b_d = np.asarray(inputs["b_d"], np.float32)
    outs = []
    for cix in range(NCORES):
        arr = res.results[cix]["y"] / SP                       # [2, PH*B]
        outs.append(arr.reshape(2, PH, B).transpose(2, 1, 0))
    y = np.concatenate(outs, axis=0) + b_d[None, None, :]
    return np.ascontiguousarray(y, dtype=np.float32), res


def kernel(**inputs):
    y, _ = run(inputs, trace=False)
    return y


# revision 9
# speedup vs baseline: 1.0988x; 1.0693x over previous
"""Trainium2 Bass kernel for the LSTM decoder — v4: latency-optimized
two-recurrence schedule.

Per core (256 batch rows) the batch is split into two independent 128-column
recurrences (X, Y) phase-shifted by half a step.  v4 restructures v3 to cut
the per-half critical path (which set the v3 period):

- PSUM bank order [i, g, f, o]: one merged sigmoid ACT over [0:1536] covers
  every gate the DVE c-chain needs (i, g, f); sigma(o) is a separate ACT that
  only gates the h-write.  Matmuls are emitted igf-banks first, o-bank after,
  so the big sigmoid starts after 48 of 64 gate matmuls.
- DVE chain uses fused scalar_tensor_tensor (4x DVE mode):
    t1 = (u_g - 0.5) * u_i          # tanh(g)=2*sigmoid(2g)-1 folded
    t2 = (ct * 1) * u_f
    ct = (t1 * 2) + t2
  3 ops instead of 4, each ~194ns instead of ~327ns.
- h8 (fp8 h for next-step matmuls) written as two [128, 256] chunks so the
  kc01 matmuls of the next step start one chunk earlier; h16 (fp16 h for y)
  is a cheap STT after; y_copy runs on the idle Pool engine.
- PE emission order is ready-time monotonic so the 4-deep wait queue never
  traps ready matmuls behind blocked ones:
    igf(h0,t) o(h0,t) | y(h1,t-2) bias(h1,t) | igf(h1,t) o(h1,t) |
    y(h0,t-1) bias(h0,t+1)
- fp8 (e4m3) DoubleRow matmuls: two K-slots carry the (hi, lo) split of the
  merged W = W_ih + W_hh (valid since the output h is fed back as the next
  input); moving operand is the plain fp8 h chunk broadcast into both slots.
- y_t = W_d @ h_t (fp16) lands in the o-bank s0 PSUM region after sigma(o)
  reads it; scaled and bias-added on the host.
"""

import numpy as np
import ml_dtypes
from contextlib import ExitStack

import concourse.bacc as bacc
import concourse.mybir as mybir
from concourse import tile
from concourse.bass_utils import run_bass_kernel_spmd

fp32 = mybir.dt.float32
fp16 = mybir.dt.float16
fp8 = mybir.dt.float8e4
F8 = ml_dtypes.float8_e4m3fn
AF = mybir.ActivationFunctionType
ALU = mybir.AluOpType
DR = mybir.MatmulPerfMode.DoubleRow

P = 128
B = 256          # batch rows per core
HW = 128         # half-batch width
HC = 4           # hidden chunks of 128
NT = 16          # gate tiles per half
PH = 32
NCORES = 8
SP = 1024.0      # weight/bias scale (keeps all fp8 <= 240: IEEE-e4m3 safe)

# gate-class order in PSUM banks: [i, g, f, o]; W row bases (torch i,f,g,o)
RB = [0, 1024, 512, 1536]

_CACHE = {}


def _build():
    nc = bacc.Bacc("TRN2", target_bir_lowering=False, debug=False,
                   num_devices=NCORES)

    ws_d = nc.dram_tensor("ws", [P, HC, 2, NT, P], fp8, kind="ExternalInput")
    wih_d = nc.dram_tensor("wih", [P, HC, 2, NT, P], fp8, kind="ExternalInput")
    bs_d = nc.dram_tensor("bs", [1, 2, NT, P], fp8, kind="ExternalInput")
    wd_d = nc.dram_tensor("wd", [P, HC, 2], fp16, kind="ExternalInput")
    zhi_d = nc.dram_tensor("zhi", [P, HC, B], fp8, kind="ExternalInput")
    zlo_d = nc.dram_tensor("zlo", [P, HC, B], fp8, kind="ExternalInput")
    ones_d = nc.dram_tensor("ones", [1, HW], fp8, kind="ExternalInput")
    y_d = nc.dram_tensor("y", [2, PH * B], fp32, kind="ExternalOutput")

    with tile.TileContext(nc) as tc:
        with ExitStack() as ctx:
            const = ctx.enter_context(tc.tile_pool(name="const", bufs=1))
            state = ctx.enter_context(tc.tile_pool(name="state", bufs=1))
            pp = ctx.enter_context(tc.tile_pool(name="pp", bufs=1,
                                                space="PSUM"))

            wih = const.tile([P, HC, 2, NT, P], fp8)
            for kc in range(HC):
                nc.sync.dma_start(wih[:, kc], wih_d[:, kc])
            zhi = const.tile([P, HC, B], fp8)
            zlo = const.tile([P, HC, B], fp8)
            nc.sync.dma_start(zhi[:], zhi_d[:])
            nc.sync.dma_start(zlo[:], zlo_d[:])
            bs = const.tile([1, 2, NT, P], fp8)
            nc.sync.dma_start(bs[:], bs_d[:])
            ones = const.tile([1, HW], fp8)
            nc.sync.dma_start(ones[:], ones_d[:])
            wd = const.tile([P, HC, 2], fp16)
            nc.sync.dma_start(wd[:], wd_d[:])
            ws = const.tile([P, HC, 2, NT, P], fp8)
            for kc in range(HC):
                nc.sync.dma_start(ws[:, kc], ws_d[:, kc])

            pH = [pp.tile([P, 2048], fp32, tag=f"p{h}", name=f"p{h}")
                  for h in range(2)]
            u = [state.tile([P, 2048], fp16, tag=f"u{h}", name=f"u{h}")
                 for h in range(2)]
            ct = [state.tile([P, 512], fp16, tag=f"c{h}", name=f"c{h}")
                  for h in range(2)]
            tct = [state.tile([P, 512], fp16, tag=f"tc{h}", name=f"tc{h}")
                   for h in range(2)]
            gt = [state.tile([P, 512], fp16, tag=f"gt{h}", name=f"gt{h}")
                  for h in range(2)]
            t1 = [state.tile([P, 512], fp16, tag=f"t1{h}", name=f"t1{h}")
                  for h in range(2)]
            t2 = [state.tile([P, 512], fp16, tag=f"t2{h}", name=f"t2{h}")
                  for h in range(2)]
            h8 = [[state.tile([P, HC * HW], fp8, tag=f"h8{h}b{b}",
                              name=f"h8{h}b{b}") for b in range(2)]
                  for h in range(2)]
            h16 = [[state.tile([P, HC * HW], fp16, tag=f"h16{h}b{b}",
                               name=f"h16{h}b{b}") for b in range(2)]
                   for h in range(2)]
            y_sb = const.tile([2, PH * B], fp32)
            # per-partition 1/SP scale tiles for the big sigmoids; rewritten
            # after the OTHER half's ct so sigma_igf(h) cannot preempt the
            # other half's tanh in the ACT queue
            sc = [state.tile([P, 1], fp32, tag=f"sc{h}", name=f"sc{h}")
                  for h in range(2)]
            ones_b = ones[:].unsqueeze(1).broadcast_to([1, 2, HW])

            def mov(src_ap):
                return src_ap.unsqueeze(1).broadcast_to([P, 2, HW])

            def tau(beta, s):
                return 4 * beta + s

            def out_ap(h, beta, s):
                c0 = 512 * beta + HW * s
                return pH[h][:, c0:c0 + HW]

            def bias_mm(h, beta, s):
                nc.tensor.matmul(out_ap(h, beta, s),
                                 bs[0:1, :, tau(beta, s), :], ones_b,
                                 start=True if s == 0 else False,
                                 stop=False, perf_mode=DR)

            def kg_mm(h, beta, s, kc, W, src_ap, stop):
                nc.tensor.matmul(out_ap(h, beta, s),
                                 W[:, kc, :, tau(beta, s), :], mov(src_ap),
                                 start=False, stop=stop, perf_mode=DR)

            def hsrc(h, t, kc):
                return h8[h][t % 2][:, kc * HW:(kc + 1) * HW]

            def bias_all(h):
                for beta in range(4):
                    for s in range(HC):
                        bias_mm(h, beta, s)

            def kg_igf(h, t):
                # i, g, f banks (betas 0,1,2), kc01 then kc23: the merged
                # sigmoid needs only these 48 matmuls
                for kcp in ((0, 1), (2, 3)):
                    for kc in kcp:
                        for beta in range(3):
                            for s in range(HC):
                                kg_mm(h, beta, s, kc, ws, hsrc(h, t - 1, kc),
                                      stop=(kc == 3 and s == HC - 1))

            def kg_o(h, t):
                for kc in range(HC):
                    for s in range(HC):
                        kg_mm(h, 3, s, kc, ws, hsrc(h, t - 1, kc),
                              stop=(kc == 3 and s == HC - 1))

            def y_mm(h, t):
                # fp16 matmul from the fp16 h copy (fp8 h noise is too big
                # for y at early steps); lands in o-bank s0 after sigma(o)
                # reads it
                out = pH[h][0:2, 1536:1536 + HW]
                for kc in range(HC):
                    nc.tensor.matmul(out, wd[:, kc, :],
                                     h16[h][t % 2][:, kc * HW:(kc + 1) * HW],
                                     start=(kc == 0), stop=(kc == 3))

            def y_copy(h, t):
                nc.gpsimd.tensor_copy(y_sb[:, B * t + HW * h:
                                           B * t + HW * (h + 1)],
                                      pH[h][0:2, 1536:1536 + HW])

            def step0_mm(h):
                # x = z (two-term), f-gate (beta=2) dead since c0 = 0
                for beta in (0, 1, 3):
                    for s in range(HC):
                        bias_mm(h, beta, s)
                for kc in range(HC):
                    zh = zhi[:, kc, HW * h:HW * (h + 1)]
                    zl = zlo[:, kc, HW * h:HW * (h + 1)]
                    for beta in (0, 1, 3):
                        for s in range(HC):
                            kg_mm(h, beta, s, kc, wih, zh, stop=False)
                            kg_mm(h, beta, s, kc, wih, zl,
                                  stop=(kc == 3 and beta == 3 and s == HC - 1))

            def act_sig(h, t):
                if t == 0:
                    nc.scalar.activation(u[h][:, 0:1024], pH[h][:, 0:1024],
                                         AF.Sigmoid, scale=sc[h][:])
                else:
                    nc.scalar.activation(u[h][:, 0:1536], pH[h][:, 0:1536],
                                         AF.Sigmoid, scale=sc[h][:])
                nc.scalar.activation(u[h][:, 1536:2048], pH[h][:, 1536:2048],
                                     AF.Sigmoid, scale=1.0 / SP)

            def dve_c(h, t):
                # gtil = 2*u_g - 1 = tanh(g)  (4x tensor_scalar)
                nc.vector.tensor_scalar(gt[h][:], u[h][:, 512:1024], 2.0, 1.0,
                                        ALU.mult, ALU.subtract)
                if t == 0:
                    nc.vector.tensor_mul(ct[h][:], u[h][:, 0:512], gt[h][:])
                else:
                    nc.vector.tensor_mul(t1[h][:], u[h][:, 0:512], gt[h][:])
                    nc.vector.tensor_mul(t2[h][:], u[h][:, 1024:1536],
                                         ct[h][:])
                    nc.vector.tensor_add(ct[h][:], t1[h][:], t2[h][:])
                # rewrite the other half's sigmoid scale (same 1/SP value):
                # orders sigma_igf(1-h) after this ct in the ACT queue
                nc.vector.tensor_scalar(sc[1 - h][:], ct[h][:, 0:1], 0.0,
                                        1.0 / SP, ALU.mult, ALU.add)

            def tanh_h(h):
                nc.scalar.activation(tct[h][:], ct[h][:], AF.Tanh)

            def hmuls(h, t):
                hb = h8[h][t % 2]
                nc.vector.tensor_mul(hb[:, 0:256], u[h][:, 1536:1792],
                                     tct[h][:, 0:256])
                nc.vector.tensor_mul(hb[:, 256:512], u[h][:, 1792:2048],
                                     tct[h][:, 256:512])
                nc.vector.tensor_mul(h16[h][t % 2][:], u[h][:, 1536:2048],
                                     tct[h][:])

            def chain(h, t):
                act_sig(h, t)
                dve_c(h, t)
                tanh_h(h)
                hmuls(h, t)

            # --- step 0 ---
            for h in range(2):
                nc.vector.tensor_scalar(sc[h][:], zhi[:, 0, 0:1], 0.0,
                                        1.0 / SP, ALU.mult, ALU.add)
            for h in range(2):
                step0_mm(h)
                chain(h, 0)
            bias_all(0)

            # --- steady steps: PE emission is ready-time monotonic ---
            # igf(h0,t) o(h0,t) | y(h1,t-2) bias(h1,t) | igf(h1,t) o(h1,t) |
            # y(h0,t-1) bias(h0,t+1)
            for t in range(1, PH):
                kg_igf(0, t)
                kg_o(0, t)
                chain(0, t)
                if t >= 2:
                    y_mm(1, t - 2)
                    y_copy(1, t - 2)
                bias_all(1)
                kg_igf(1, t)
                kg_o(1, t)
                chain(1, t)
                y_mm(0, t - 1)
                y_copy(0, t - 1)
                if t < PH - 1:
                    bias_all(0)

            # --- drain the y tail ---
            y_mm(1, PH - 2)
            y_copy(1, PH - 2)
            y_mm(0, PH - 1)
            y_copy(0, PH - 1)
            y_mm(1, PH - 1)
            y_copy(1, PH - 1)
            nc.sync.dma_start(y_d[:], y_sb[:])
    nc.compile()
    return nc


def _get_nc():
    if "nc" not in _CACHE:
        _CACHE["nc"] = _build()
    return _CACHE["nc"]


def _enc8(x):
    return np.asarray(F8(np.asarray(x, np.float32)))


def _prep_inputs(z, W_ih, W_hh, b_ih, b_hh, W_d):
    z2 = np.asarray(z, np.float32).reshape(2048, 512)
    W_ih = np.asarray(W_ih, np.float32)
    W_sum = W_ih + np.asarray(W_hh, np.float32)
    bias = (np.asarray(b_ih, np.float32) + np.asarray(b_hh, np.float32))

    def fold_w(W):
        W2 = W * SP
        W2[1024:1536] *= 2.0
        return W2

    W2 = fold_w(W_sum)
    Wih2 = fold_w(W_ih)
    Bp = bias * SP
    Bp[1024:1536] *= 2.0

    # tile tau = 4*beta + s -> W rows RB[beta] + 128*s
    rows = np.empty((NT, P), np.int64)
    for beta in range(4):
        for s in range(HC):
            rows[4 * beta + s] = RB[beta] + 128 * s + np.arange(P)

    def to_ws(W2f):
        hi = _enc8(W2f).astype(np.float32)
        lo = _enc8(W2f - hi).astype(np.float32)

        def lay(Wq):
            a = Wq[rows]                                      # [16,128,512]
            a = a.reshape(NT, P, HC, P).transpose(3, 2, 0, 1)  # [p,kc,tau,m]
            return a
        out = np.stack([lay(hi), lay(lo)], axis=2)             # [p,kc,2,tau,m]
        return np.ascontiguousarray(_enc8(out))

    ws = to_ws(W2)
    wih = to_ws(Wih2)

    bhi = _enc8(Bp).astype(np.float32)
    blo = _enc8(Bp - bhi).astype(np.float32)
    bs = np.stack([bhi[rows], blo[rows]], axis=0)
    bs = np.ascontiguousarray(_enc8(bs[None]))                 # [1,2,16,128]

    Wd2 = np.asarray(W_d, np.float32) * SP
    wd = np.ascontiguousarray(
        Wd2.T.reshape(HC, P, 2).transpose(1, 0, 2)).astype(np.float16)

    ones = _enc8(np.ones((1, HW), np.float32))

    in_maps = []
    for cix in range(NCORES):
        zc = z2[cix * B:(cix + 1) * B].T                       # [512, 256]
        zhi = _enc8(zc).astype(np.float32)
        zlo = _enc8(zc - zhi).astype(np.float32)
        zhi = zhi.reshape(HC, P, B).transpose(1, 0, 2)
        zlo = zlo.reshape(HC, P, B).transpose(1, 0, 2)
        in_maps.append({
            "ws": ws, "wih": wih, "bs": bs, "wd": wd,
            "zhi": np.ascontiguousarray(_enc8(zhi)),
            "zlo": np.ascontiguousarray(_enc8(zlo)),
            "ones": ones,
        })
    return in_maps


def run(inputs, trace=False, **kw):
    nc = _get_nc()
    in_maps = _prep_inputs(inputs["z"], inputs["W_ih"], inputs["W_hh"],
                           inputs["b_ih"], inputs["b_hh"], inputs["W_d"])
    res = run_bass_kernel_spmd(nc, in_maps, core_ids=list(range(NCORES)),
                               trace=trace, **kw)
    b_d = np.asarray(inputs["b_d"], np.float32)
    outs = []
    for cix in range(NCORES):
        arr = res.results[cix]["y"] / SP                       # [2, PH*B]
        outs.append(arr.reshape(2, PH, B).transpose(2, 1, 0))
    y = np.concatenate(outs, axis=0) + b_d[None, None, :]
    return np.ascontiguousarray(y, dtype=np.float32), res


def kernel(**inputs):
    y, _ = run(inputs, trace=False)
    return y
